# revision 6
# baseline (speedup 1.0000x reference)
"""Conditional per-sample 64x64 matmul (MoE-style routing), Trainium2 Bass kernel.

out[b, d, t] = sum_c x[b, c, t] * weights[cond_ids[b], c, d]

Strategy:
  - Host gathers the per-sample weight [B, Cin, Cout] (tiny) and packs
    adjacent sample pairs into block-diagonal [128, 128] stationary
    matrices so each matmul uses all 128 PE rows / SBUF partitions.
  - Data-parallel across 8 NeuronCores over the batch: 16 samples
    (= 8 pairs) per core.
  - Per pair: x slice is a [128, 8192] f32 view (2 samples x 64 chans).
    Stream T in chunks of 4096: DMA 2MiB in -> 2x (4 matmuls (K=128,
    N=512) into a 4-bank PSUM tile -> DVE copy to SBUF) -> DMA 2MiB out.
  - Executed through the same bass_exec/PJRT path run_bass_kernel_spmd
    uses under axon, but with the jitted executable cached so repeated
    kernel() calls don't re-trace/re-compile.
"""

import numpy as np

import jax
import jax.numpy as jnp
from jax.experimental.shard_map import shard_map
from jax.sharding import Mesh, NamedSharding, PartitionSpec

import concourse.bacc as bacc
import concourse.bass as bass
import concourse.mybir as mybir
import concourse.tile as tile
from concourse.bass2jax import (
    _bass_exec_p,
    install_neuronx_cc_hook,
    partition_id_tensor,
)

B = 128
CIN = 64
COUT = 64
T = 8192
NCORES = 8
PAIRS = B // 2                   # 64 sample pairs
PPC = PAIRS // NCORES            # 8 pairs per core
CHUNK = 4096                     # T chunk per DMA (2 MiB tiles)
MMFREE = 512                     # matmul free dim (one PSUM bank, fp32)

_NC_CACHE = {}
_RUNNER_CACHE = {}
_ZEROS = {}

import ml_dtypes

NP_DT = {"f32": np.float32, "f16": np.float16, "bf16": ml_dtypes.bfloat16}

# Best measured config: fp16 I/O halves HBM traffic (the roofline) at
# 3.7e-4 rel err; each two-pair group loads as ONE fused DMA (pairs are
# DRAM-adjacent; long same-direction bursts cut HBM R/W turnaround),
# 4-bank PSUM tiles, DVE copies (cast f32->f16), single sync HWDGE ring.
BEST_KW = dict(chunk=4096, xbufs=2, obufs=3, bigload="fused", group=2, iodt="f16")


def _build_nc(
    reps: int = 1,
    chunk: int = CHUNK,
    xbufs: int = 3,
    obufs: int = 3,
    load_eng: str = "sync",
    store_eng: str = "sync",
    compute: bool = True,
    pschunk: int = 2048,
    copy_alt: bool = False,
    wconsol: bool = False,
    store_split: bool = False,
    bigload: bool = False,
    group: int = 1,  # pairs loaded back-to-back before their stores (bigload only)
    dma_mode: str = "both",  # for compute=False: "both" | "load" | "store"
    w_eng: str | None = None,  # ring for weight loads (default: load_eng)
    w_group: bool = False,  # issue the whole group's weight loads first
    iodt: str = "f32",  # DRAM/SBUF dtype for x/w/out: "f32" | "f16" | "bf16"
):
    f32 = mybir.dt.float32
    dt = {"f32": f32, "f16": mybir.dt.float16, "bf16": mybir.dt.bfloat16}[iodt]
    nc = bacc.Bacc("TRN2", target_bir_lowering=False, debug=False)

    x_d = nc.dram_tensor("x", [PPC, 128, T], dt, kind="ExternalInput").ap()
    w_d = nc.dram_tensor("wp", [PPC, 128, 128], dt, kind="ExternalInput").ap()
    o_d = nc.dram_tensor("out", [PPC, 128, T], dt, kind="ExternalOutput").ap()

    ld = getattr(nc, load_eng)
    st = getattr(nc, store_eng)

    with tile.TileContext(nc) as tc:
        with (
            tc.tile_pool(name="wpool", bufs=(2 * group + 2) if w_group else 2) as wpool,
            tc.tile_pool(name="xpool", bufs=xbufs) as xpool,
            tc.tile_pool(name="opool", bufs=obufs) as opool,
            tc.tile_pool(name="pspool", bufs=2, space=bass.MemorySpace.PSUM) as pspool,
        ):
            if not compute and dma_mode == "store":
                # store-only: stream one preset SBUF tile to every out slice
                seed_t = xpool.tile([128, chunk], dt, tag="seed")
                nc.vector.memset(seed_t[:], 1.0)
            for _ in range(reps):
                if compute and wconsol:
                    w_all = wpool.tile([128, PPC, 128], dt)
                    ld.dma_start(out=w_all[:], in_=w_d.rearrange("p q c -> q p c"))
                group_tiles = {}
                chunk_tiles = {}
                w_tiles = {}
                for p in range(PPC):
                    if compute and not wconsol:
                        if w_group:
                            if p % group == 0:
                                for q in range(p, min(p + group, PPC)):
                                    wq_t = wpool.tile([128, 128], dt)
                                    getattr(nc, w_eng or load_eng).dma_start(
                                        out=wq_t[:], in_=w_d[q]
                                    )
                                    w_tiles[q] = wq_t
                            w_t = w_tiles.pop(p)
                        else:
                            w_t = wpool.tile([128, 128], dt)
                            getattr(nc, w_eng or load_eng).dma_start(
                                out=w_t[:], in_=w_d[p]
                            )
                    elif compute:
                        w_t = w_all[:, p]
                    if bigload == "fused":
                        # one DMA for the whole group: pairs are adjacent in
                        # DRAM, so [group*4MiB] moves as a single transfer
                        if p % group == 0:
                            xg_t = xpool.tile([128, group, T], dt)
                            ld.dma_start(
                                out=xg_t[:],
                                in_=x_d[p : p + group].rearrange("p q t -> q p t"),
                            )
                            for qi in range(group):
                                group_tiles[p + qi] = xg_t[:, qi]
                        xp_t = group_tiles.pop(p)
                    elif bigload:
                        if p % group == 0:
                            for q in range(p, min(p + group, PPC)):
                                xq_t = xpool.tile([128, T], dt)
                                ld.dma_start(out=xq_t[:], in_=x_d[q])
                                group_tiles[q] = xq_t
                        xp_t = group_tiles.pop(p)
                    elif group > 1 and p % group == 0:
                        # chunked group-batch: issue all of the group's chunk
                        # loads back-to-back for long same-direction bursts
                        for q in range(p, min(p + group, PPC)):
                            for j in range(T // chunk):
                                t = xpool.tile([128, chunk], dt)
                                ld.dma_start(
                                    out=t[:],
                                    in_=x_d[q, :, j * chunk : (j + 1) * chunk],
                                )
                                chunk_tiles[(q, j)] = t
                    for j in range(T // chunk):
                        if bigload:
                            x_t = xp_t[:, j * chunk : (j + 1) * chunk]
                        elif group > 1:
                            x_t = chunk_tiles.pop((p, j))
                        elif compute or dma_mode in ("both", "load"):
                            x_t = xpool.tile([128, chunk], dt)
                            ld.dma_start(
                                out=x_t[:], in_=x_d[p, :, j * chunk : (j + 1) * chunk]
                            )
                        if compute:
                            o_t = opool.tile([128, chunk], dt)
                            for h in range(chunk // pschunk):
                                ps_t = pspool.tile([128, pschunk], f32)
                                for k in range(pschunk // MMFREE):
                                    c0 = k * MMFREE
                                    nc.tensor.matmul(
                                        ps_t[:, c0 : c0 + MMFREE],
                                        w_t[:],
                                        x_t[:, h * pschunk + c0 : h * pschunk + c0 + MMFREE],
                                    )
                                dst = o_t[:, h * pschunk : (h + 1) * pschunk]
                                if copy_alt and (j * 8 + h) % 2:
                                    nc.scalar.copy(dst, ps_t[:])
                                else:
                                    nc.vector.tensor_copy(dst, ps_t[:])
                                if store_split:
                                    t0 = j * chunk + h * pschunk
                                    st.dma_start(
                                        out=o_d[p, :, t0 : t0 + pschunk], in_=dst
                                    )
                            src = o_t
                        elif dma_mode == "load":
                            # tiny consumer so dead-code passes keep the loads
                            o_t = opool.tile([128, 128], dt)
                            nc.vector.tensor_copy(o_t[:], x_t[:, :128])
                            st.dma_start(out=o_d[p, :, :128], in_=o_t[:])
                            continue
                        elif dma_mode == "store":
                            src = seed_t
                        else:
                            src = x_t
                        if not (compute and store_split):
                            st.dma_start(
                                out=o_d[p, :, j * chunk : (j + 1) * chunk], in_=src[:]
                            )
    nc.compile()
    return nc


def _get_nc(reps: int = 1, **kw):
    key = (reps, tuple(sorted(kw.items())))
    if key not in _NC_CACHE:
        _NC_CACHE[key] = _build_nc(reps, **kw)
    return _NC_CACHE[key]


def make_runner(reps: int = 1, **kw):
    """Jitted sharded executable for the bass program; cached across calls.

    Takes global arrays x_pairs [PAIRS,128,T], wp [PAIRS,128,128],
    zeros [PAIRS,128,T]; returns global out [PAIRS,128,T].
    Mirrors concourse.bass2jax.run_bass_via_pjrt's multi-core path
    (operands must be jit parameters, in order, for neuronx_cc_hook).
    """
    key = (reps, tuple(sorted(kw.items())))
    if key in _RUNNER_CACHE:
        return _RUNNER_CACHE[key]
    install_neuronx_cc_hook()
    nc = _get_nc(reps, **kw)
    np_dt = NP_DT[kw.get("iodt", "f32")]
    out_aval = jax.core.ShapedArray((PPC, 128, T), np_dt)

    def _body(x, wp, z):
        outs = _bass_exec_p.bind(
            x,
            wp,
            z,
            partition_id_tensor(),
            out_avals=(out_aval,),
            in_names=("x", "wp", "out", "partition_id"),
            out_names=("out",),
            lowering_input_output_aliases=(),
            sim_require_finite=True,
            sim_require_nnan=True,
            nc=nc,
        )
        return outs[0]

    devices = jax.devices()[:NCORES]
    mesh = Mesh(np.asarray(devices), ("core",))
    spec = PartitionSpec("core")
    fn = jax.jit(
        shard_map(
            _body,
            mesh=mesh,
            in_specs=(spec, spec, spec),
            out_specs=spec,
            check_rep=False,
        )
    )
    _RUNNER_CACHE[key] = (fn, mesh)
    return fn, mesh


def _get_zeros(mesh, np_dt=np.float32):
    # Device-resident, sharded zero buffer for the NEFF "out" input slot.
    # The kernel overwrites every element, so contents are irrelevant and
    # the buffer can be reused across calls (never donated).
    if np_dt not in _ZEROS:
        sharding = NamedSharding(mesh, PartitionSpec("core"))
        _ZEROS[np_dt] = jax.jit(
            lambda: jnp.zeros((PAIRS, 128, T), np_dt),
            out_shardings=sharding,
        )()
    return _ZEROS[np_dt]


def kernel(x: np.ndarray, weights: np.ndarray, cond_ids: np.ndarray) -> np.ndarray:
    x = np.ascontiguousarray(np.asarray(x, dtype=np.float32))
    weights = np.asarray(weights, dtype=np.float32)
    cond_ids = np.asarray(cond_ids, dtype=np.int32)
    np_dt = NP_DT[BEST_KW.get("iodt", "f32")]

    # Host-side routing: gather per-sample weights, pack sample pairs into
    # block-diagonal [128, 128] stationary matrices.
    w_full = weights[cond_ids]                      # [B, CIN, COUT]
    wp = np.zeros((PAIRS, 2 * CIN, 2 * COUT), dtype=np.float32)
    wp[:, :CIN, :COUT] = w_full[0::2]
    wp[:, CIN:, COUT:] = w_full[1::2]

    x_pairs = x.reshape(PAIRS, 2 * CIN, T).astype(np_dt, copy=False)
    wp = wp.astype(np_dt, copy=False)

    fn, mesh = make_runner(reps=1, **BEST_KW)
    out = fn(x_pairs, wp, _get_zeros(mesh, np_dt))
    return np.asarray(out).astype(np.float32, copy=False).reshape(B, COUT, T)

